# revision 1
# baseline (speedup 1.0000x reference)
"""Differential attention (B=1, N=2048, C=1024, H=16) on 8 Trainium2 NeuronCores.

Sharding: tensor-parallel over heads. Each core owns 2 heads: it computes the
QKV projection for its heads only, runs RoPE + the two softmaxes + PV locally,
then the per-head attention outputs (in transposed [dv, n] layout) are
AllGathered chunk-by-chunk (4 x 256KB/rank, far cheaper than the 8MB
all-reduce alternative and overlapped with the attention tail) and each core
computes a 128-column slice of the output projection.

All matmuls run in bf16 with fp32 PSUM accumulation. Softmax skips
max-subtraction: logits are q.k/8 with q,k ~ N(0,1) so |logit| < ~7 and exp()
is comfortably inside the fp32 range. Denominators are accumulated in bf16 on
the DVE (2-byte fast mode) and reduced across partitions on GPSIMD in fp32.

Schedule notes (cost-model-driven):
- Inputs load via FEW large DMAs: each dma_start costs ~650ns of SP-sequencer
  issue + ~625ns of shared-HWDGE time regardless of size, so per-tile loads
  serialize the prologue on instruction issue, not bytes.
- The PE clock p-state ramps only while the engine stays busy; idle gaps
  restart the following instruction train at 0.65-1.2GHz. The prologue is
  ordered so the PE never starves (v strips interleaved between strip groups,
  filler tasks front-load rope-independent work).
- The final chunk is processed as head-0 full width plus two 256-query head-1
  sub-units, each with its own small AllGather, so the exposed tail after the
  last PV is one quarter-size gather + 8 matmuls instead of a half-chunk
  chain.
"""

import os
import sys

import numpy as np
import ml_dtypes

for _p in ("/opt/trn_rl_repo", os.path.expanduser("~/.axon_site/_ro/trn_rl_repo")):
    if os.path.isdir(_p) and _p not in sys.path:
        sys.path.insert(0, _p)

import concourse.bass as bass  # noqa: E402
import concourse.tile as tile  # noqa: E402
from concourse import bacc, bass_isa, mybir  # noqa: E402
from concourse.alu_op_type import AluOpType  # noqa: E402
from concourse.bass_utils import run_bass_kernel_spmd  # noqa: E402

BF16 = ml_dtypes.bfloat16
B, N, C = 1, 2048, 1024
H = 16
HD = 64  # head dim of each rope/attn half
DV = 2 * HD  # value dim per head (128)
NCORES = 8
HPC = H // NCORES  # heads per core = 2
P = 128
KT = C // P  # 8 contraction tiles for QKV
JT = N // P  # 16 key-position tiles
NCH = 4  # 512-wide chunks of the sequence
CH = N // NCH  # 512
SCALE = HD**-0.5

_PROG = None


def _build_program(collective=True):
    dt = mybir.dt
    f32 = dt.float32
    bf = dt.bfloat16
    Exp = mybir.ActivationFunctionType.Exp

    nc = bacc.Bacc(
        "TRN2", target_bir_lowering=False, debug=False, num_devices=NCORES
    )

    xT = nc.dram_tensor("xT", [C, N], bf, kind="ExternalInput")
    wqk = nc.dram_tensor("wqk", [C, 4 * P], bf, kind="ExternalInput")
    wv = nc.dram_tensor("wv", [C, HPC * DV], bf, kind="ExternalInput")
    wpj = nc.dram_tensor("wpj", [2 * C, P], bf, kind="ExternalInput")
    csT = nc.dram_tensor("csT", [2 * P, N], bf, kind="ExternalInput")
    nlam = nc.dram_tensor("nlam", [1, 1], f32, kind="ExternalInput")
    yT = nc.dram_tensor("yT", [P, N], f32, kind="ExternalOutput")

    with tile.TileContext(nc) as tc:
        with tc.tile_pool(name="const", bufs=1) as const, tc.tile_pool(
            name="dram", bufs=1, space="DRAM"
        ) as dram:
            # ---- persistent SBUF tensors ----
            # Inputs are loaded with FEW large DMAs: every dma_start costs
            # ~650ns on the SP sequencer + ~625ns on the shared HWDGE
            # regardless of size, so per-[128,512]-tile loads serialize the
            # prologue on instruction issue, not bytes. Chunk 0 of x and wqk
            # are split into k-pair pieces so the first QKV matmul starts
            # after ~0.5MB; everything else is one DMA per tensor.
            wqk_all = const.tile([P, KT, 4 * P], bf, tag="wqk", name="wqk")
            x0 = const.tile([P, KT, CH], bf, tag="xc0", name="xc0")
            for kp in range(4):
                ks = slice(2 * kp * P, 2 * (kp + 1) * P)
                nc.sync.dma_start(
                    wqk_all[:, 2 * kp : 2 * kp + 2, :],
                    wqk.ap()[ks, :].rearrange("(t p) n -> p t n", p=P),
                )
                nc.sync.dma_start(
                    x0[:, 2 * kp : 2 * kp + 2, :],
                    xT.ap()[ks, 0:CH].rearrange("(t p) n -> p t n", p=P),
                )
            wqk_sb = [wqk_all[:, k, :] for k in range(KT)]
            xc = [x0] + [
                const.tile([P, KT, CH], bf, tag=f"xc{c}", name=f"xc{c}")
                for c in range(1, NCH)
            ]
            xsb = [[xc[c][:, k, :] for c in range(NCH)] for k in range(KT)]

            def load_x_chunk(c):
                nc.sync.dma_start(
                    xc[c],
                    xT.ap()[:, c * CH : (c + 1) * CH].rearrange(
                        "(t p) n -> p t n", p=P
                    ),
                )

            cs_sb = const.tile([P, 2, N], bf, tag="cs", name="cs")
            nc.sync.dma_start(cs_sb, csT.ap().rearrange("(t p) n -> p t n", p=P))
            cos_sb = cs_sb[:, 0, :]
            sin_sb = cs_sb[:, 1, :]
            load_x_chunk(1)
            wv_all = const.tile([P, KT, HPC * DV], bf, tag="wv", name="wv")
            nc.sync.dma_start(wv_all, wv.ap().rearrange("(t p) n -> p t n", p=P))
            wv_sb = [wv_all[:, k, :] for k in range(KT)]
            load_x_chunk(2)
            load_x_chunk(3)
            nlam_sb = const.tile([1, 1], f32, tag="nlam")
            nc.sync.dma_start(nlam_sb, nlam.ap())
            nlam_bc = const.tile([P, 1], f32, tag="nlambc")
            nc.gpsimd.partition_broadcast(nlam_bc, nlam_sb)
            wpj_all = const.tile([P, JT, P], bf, tag="wpj", name="wpj")
            nc.sync.dma_start(wpj_all, wpj.ap().rearrange("(t p) n -> p t n", p=P))
            wpj_sb = [wpj_all[:, k, :] for k in range(JT)]

            # rope'd projections, [d(2 heads stacked), n], one tile per chunk
            def chunk_tiles(nm):
                return [
                    const.tile([P, CH], bf, tag=f"{nm}{c}", name=f"{nm}{c}")
                    for c in range(NCH)
                ]

            q1c, q2c = chunk_tiles("q1c"), chunk_tiles("q2c")
            k1c, k2c = chunk_tiles("k1c"), chunk_tiles("k2c")
            # values in [j, dv] layout, per head, split into 4 j-groups so
            # PV can start before the whole V projection has finished
            vsb = [
                [
                    const.tile([P, 4, DV], bf, tag=f"vsb{h}_{g}", name=f"vsb{h}_{g}")
                    for g in range(4)
                ]
                for h in range(HPC)
            ]

            # per-chunk AllGather buffers (overlap collectives w/ compute)
            ag_in = [
                dram.tile([HPC * DV, CH], bf, name=f"ag_in{i}") for i in range(NCH)
            ]
            ag_out = [
                dram.tile([2 * C, CH], bf, addr_space="Shared", name=f"ag_out{i}")
                for i in range(NCH)
            ]
            # final chunk gathered in three pieces so the exposed tail after
            # the very last PV is a quarter-size gather + 8 matmuls: head 0
            # full-width (fires two units early), head 1 as two 256-query
            # sub-units
            HW2 = CH // 2
            ag_out3h0 = dram.tile([C, CH], bf, addr_space="Shared", name="ag_out3h0")
            ag3q_in = [
                dram.tile([P, HW2], bf, name=f"ag3q_in{s}") for s in range(2)
            ]
            ag_out3q = [
                dram.tile([C, HW2], bf, addr_space="Shared", name=f"ag_out3q{s}")
                for s in range(2)
            ]

            # ---- phase 1 prologue: keys, q-chunk 0, first V j-group ----
            def emit_v(nb, pool, tag):
                c, off = nb // 4, (nb % 4) * P
                pvs = pool.tile([P, HPC * DV], f32, tag=tag, name=f"vstrip{nb}")
                for k in range(KT):
                    nc.tensor.matmul(
                        pvs,
                        lhsT=xsb[k][c][:, off : off + P],
                        rhs=wv_sb[k],
                        start=(k == 0),
                        stop=(k == KT - 1),
                    )
                # copies on DVE: the Activation engine is the per-pair pacer
                # (two exps), so unit-boundary copy bursts must stay off it
                for h in range(HPC):
                    nc.vector.tensor_copy(
                        vsb[h][nb // 4][:, nb % 4, :], pvs[:, h * DV : (h + 1) * DV]
                    )

            def emit_qk_strip2(m1, m2, nci, pool, tag):
                # two strips with the k-loop interleaved, so during the
                # prologue each matmul only waits for its own k-pair DMA
                pt1 = pool.tile([P, CH], f32, tag=tag, name=f"qkstrip{m1}_{nci}")
                pt2 = pool.tile([P, CH], f32, tag=tag, name=f"qkstrip{m2}_{nci}")
                for k in range(KT):
                    nc.tensor.matmul(
                        pt1,
                        lhsT=wqk_sb[k][:, m1 * P : (m1 + 1) * P],
                        rhs=xsb[k][nci],
                        start=(k == 0),
                        stop=(k == KT - 1),
                    )
                    nc.tensor.matmul(
                        pt2,
                        lhsT=wqk_sb[k][:, m2 * P : (m2 + 1) * P],
                        rhs=xsb[k][nci],
                        start=(k == 0),
                        stop=(k == KT - 1),
                    )
                return pt1, pt2

            def emit_rope(s1t, s2t, o1, o2, nci, pool):
                # ops ordered so each PSUM strip is read by exactly the first
                # two DVE ops touching it, releasing its slot early. The four
                # mults read f32 PSUM (1x DVE); intermediates land in bf16
                # SBUF so the two combine ops run in the fast 2-byte mode.
                ns = slice(nci * CH, (nci + 1) * CH)
                a = pool.tile([P, CH], bf, tag="ropetmp", name="rt_a")
                b2 = pool.tile([P, CH], bf, tag="ropetmp", name="rt_b2")
                nc.vector.tensor_tensor(a, s1t, cos_sb[:, ns], AluOpType.mult)
                nc.vector.tensor_tensor(b2, s1t, sin_sb[:, ns], AluOpType.mult)
                b = pool.tile([P, CH], bf, tag="ropetmp", name="rt_b")
                a2 = pool.tile([P, CH], bf, tag="ropetmp", name="rt_a2")
                nc.vector.tensor_tensor(b, s2t, sin_sb[:, ns], AluOpType.mult)
                nc.vector.tensor_tensor(a2, s2t, cos_sb[:, ns], AluOpType.mult)
                nc.vector.tensor_tensor(o1, a, b, AluOpType.subtract)
                nc.vector.tensor_tensor(o2, a2, b2, AluOpType.add)

            rope_cm = tc.tile_pool(name="rope", bufs=8)
            rope_t = rope_cm.__enter__()
            with tc.tile_pool(name="p_pro", bufs=6, space="PSUM") as p_pro, \
                 tc.tile_pool(name="p_prov", bufs=2, space="PSUM") as p_prov:
                # unit-0 pair p only needs key chunk p//2: rope k0/k1 here,
                # push k2/k3 into pipeline filler tasks. V strips are
                # interleaved with the k-c1 strips so the PE never waits on
                # the v-copy (Act) round-trips between p_prov slot reuses.
                sk1, sk2 = emit_qk_strip2(2, 3, 0, p_pro, "prostrip")
                emit_rope(sk1, sk2, k1c[0], k2c[0], 0, rope_t)
                sq1, sq2 = emit_qk_strip2(0, 1, 0, p_pro, "prostrip")
                emit_rope(sq1, sq2, q1c[0], q2c[0], 0, rope_t)
                emit_v(0, p_prov, "vpro")
                emit_v(1, p_prov, "vpro")
                sk1, sk2 = emit_qk_strip2(2, 3, 1, p_pro, "prostrip")
                emit_rope(sk1, sk2, k1c[1], k2c[1], 1, rope_t)
                emit_v(2, p_prov, "vpro")
                emit_v(3, p_prov, "vpro")

            # ---- phase 2: flat attention pipeline + gathered projection ----
            # Remaining V j-groups and q-chunks 1-3 are injected as filler
            # tasks into the early pipeline slots.
            with tc.tile_pool(name="e", bufs=2) as e_pool, \
                 tc.tile_pool(name="red", bufs=2) as red, \
                 tc.tile_pool(name="comb", bufs=2) as comb, \
                 tc.tile_pool(name="agst", bufs=2) as agst, \
                 tc.tile_pool(name="prhs", bufs=4) as prhs, \
                 tc.tile_pool(name="yst", bufs=1) as yst, \
                 tc.tile_pool(name="p_s", bufs=2, space="PSUM") as p_s, \
                 tc.tile_pool(name="p_pv", bufs=4, space="PSUM") as p_pv:

                def emit_allgather(ic):
                    if collective:
                        nc.gpsimd.collective_compute(
                            "AllGather",
                            AluOpType.bypass,
                            replica_groups=[list(range(NCORES))],
                            ins=[ag_in[ic][:, :]],
                            outs=[ag_out[ic][:, :]],
                        )
                    else:
                        # timing-only stand-in (single-core TimelineSim)
                        nc.sync.dma_start(ag_out[ic][: HPC * DV, :], ag_in[ic][:, :])

                proj_state = {}

                def emit_proj_dma(ic):
                    rchs = []
                    for g in range(2):
                        rch = prhs.tile(
                            [P, 8, CH], bf, tag="rch", name=f"rch{ic}_{g}", bufs=2
                        )
                        nc.sync.dma_start(
                            rch,
                            ag_out[ic][g * 8 * P : (g + 1) * 8 * P, :].rearrange(
                                "(t p) n -> p t n", p=P
                            ),
                        )
                        rchs.append(rch)
                    proj_state[ic] = rchs

                def emit_proj_mm(ic):
                    py = p_pv.tile([P, CH], f32, tag="pv", name=f"py{ic}")
                    rchs = proj_state.pop(ic)
                    for g in range(2):
                        for t in range(8):
                            kt = g * 8 + t
                            nc.tensor.matmul(
                                py, lhsT=wpj_sb[kt], rhs=rchs[g][:, t, :],
                                start=(kt == 0), stop=(kt == JT - 1),
                            )
                    ysb = yst.tile([P, CH], f32, tag="ysb", name=f"ysb{ic}")
                    nc.vector.tensor_copy(ysb, py)
                    nc.sync.dma_start(yT.ap()[:, ic * CH : (ic + 1) * CH], ysb)

                # --- final-chunk projection, in three pieces ---
                # rows r*128:(r+1)*128 of a gathered head-h buffer hold head
                # (2r + h) of the concat dim, i.e. global kt = 2r + h
                py3 = [None]
                p3st = {}

                def emit_proj3h0_dma():
                    rch = prhs.tile(
                        [P, KT, CH], bf, tag="rch", name="rch3h0", bufs=2
                    )
                    nc.sync.dma_start(
                        rch, ag_out3h0[:, :].rearrange("(t p) n -> p t n", p=P)
                    )
                    p3st["h0"] = rch

                def emit_proj3h0_mm():
                    py3[0] = p_pv.tile([P, CH], f32, tag="pv", name="py3")
                    rch = p3st.pop("h0")
                    for r in range(KT):
                        nc.tensor.matmul(
                            py3[0], lhsT=wpj_sb[2 * r], rhs=rch[:, r, :],
                            start=(r == 0), stop=False,
                            skip_group_check=True,
                        )

                def emit_proj3h1_dma(s):
                    rch = prhs.tile(
                        [P, KT, HW2], bf, tag="rch3q", name=f"rch3q{s}", bufs=2
                    )
                    nc.sync.dma_start(
                        rch, ag_out3q[s][:, :].rearrange("(t p) n -> p t n", p=P)
                    )
                    p3st[s] = rch

                def emit_proj3h1_mm(s):
                    py = py3[0]
                    rch = p3st.pop(s)
                    cols = slice(s * HW2, (s + 1) * HW2)
                    for r in range(KT):
                        nc.tensor.matmul(
                            py[:, cols], lhsT=wpj_sb[2 * r + 1], rhs=rch[:, r, :],
                            start=False, stop=(r == KT - 1),
                            skip_group_check=True,
                        )
                    ysb = yst.tile([P, HW2], f32, tag="ysb3", name=f"ysb3_{s}")
                    nc.vector.tensor_copy(ysb, py[:, cols])
                    nc.sync.dma_start(
                        yT.ap()[:, (NCH - 1) * CH + s * HW2 : (NCH - 1) * CH + (s + 1) * HW2],
                        ysb,
                    )

                UNITS = [
                    (ic, hh, 0, CH) for ic in range(NCH - 1) for hh in range(HPC)
                ] + [(3, 0, 0, CH), (3, 1, 0, HW2), (3, 1, HW2, HW2)]
                NU = len(UNITS)
                NPAIR = JT // 2
                st = [None] * NU

                def unit_alloc(u):
                    st[u] = dict(
                        e1=e_pool.tile([P, JT, CH], bf, tag="e1", name=f"e1_{u}"),
                        e2=e_pool.tile([P, JT, CH], bf, tag="e2", name=f"e2_{u}"),
                        pv1=p_pv.tile([P, CH], f32, tag="pv", name=f"pv1_{u}"),
                        pv2=p_pv.tile([P, CH], f32, tag="pv", name=f"pv2_{u}"),
                        # bf16 accumulators: 2-byte SBUF operands run the DVE
                        # adds in fast mode; the 128-partial reduce that
                        # follows washes out the rounding
                        acc1=red.tile([P, CH], bf, tag="acc1", name=f"acc1_{u}"),
                        acc2=red.tile([P, CH], bf, tag="acc2", name=f"acc2_{u}"),
                    )

                def emit_s(u, p):
                    ic, hh, q0, w = UNITS[u]
                    hs = slice(HD * hh, HD * (hh + 1))
                    qs = slice(q0, q0 + w)
                    jb = 2 * p
                    c, o0, o1 = jb // 4, (jb % 4) * P, (jb % 4 + 1) * P
                    e1, e2 = st[u]["e1"], st[u]["e2"]
                    s1 = p_s.tile([P, 2, CH], f32, tag="s", name="s1t")
                    nc.tensor.matmul(
                        s1[:, 0, :w], lhsT=k1c[c][hs, o0 : o0 + P],
                        rhs=q1c[ic][hs, qs], start=True, stop=True,
                    )
                    nc.tensor.matmul(
                        s1[:, 1, :w], lhsT=k1c[c][hs, o1 : o1 + P],
                        rhs=q1c[ic][hs, qs], start=True, stop=True,
                    )
                    nc.scalar.activation(
                        e1[:, jb : jb + 2, :w], s1[:, :, :w], Exp, scale=SCALE
                    )
                    s2 = p_s.tile([P, 2, CH], f32, tag="s", name="s2t")
                    nc.tensor.matmul(
                        s2[:, 0, :w], lhsT=k2c[c][hs, o0 : o0 + P],
                        rhs=q2c[ic][hs, qs], start=True, stop=True,
                    )
                    nc.tensor.matmul(
                        s2[:, 1, :w], lhsT=k2c[c][hs, o1 : o1 + P],
                        rhs=q2c[ic][hs, qs], start=True, stop=True,
                    )
                    nc.scalar.activation(
                        e2[:, jb : jb + 2, :w], s2[:, :, :w], Exp, scale=SCALE
                    )

                def emit_acc(u, p):
                    # softmax denominators: E1 strips summed on DVE, E2 split
                    # DVE/GPSIMD (Pool adds are ~1.9x slower than DVE), one
                    # pair behind the exps
                    e1, e2 = st[u]["e1"], st[u]["e2"]
                    acc1, acc2 = st[u]["acc1"], st[u]["acc2"]
                    _, _, _, w = UNITS[u]
                    jb = 2 * p
                    e2_eng = nc.vector if (p in (0, 3, 4, 5, 6, 7) or u >= NU - 2) else nc.gpsimd
                    if p == 0:
                        nc.vector.tensor_tensor(
                            acc1[:, :w], e1[:, 0, :w], e1[:, 1, :w], AluOpType.add
                        )
                        e2_eng.tensor_tensor(
                            acc2[:, :w], e2[:, 0, :w], e2[:, 1, :w], AluOpType.add
                        )
                    else:
                        for j in (jb, jb + 1):
                            nc.vector.tensor_tensor(
                                acc1[:, :w], acc1[:, :w], e1[:, j, :w], AluOpType.add
                            )
                            e2_eng.tensor_tensor(
                                acc2[:, :w], acc2[:, :w], e2[:, j, :w], AluOpType.add
                            )

                def finish_acc(u):
                    acc1, acc2 = st[u]["acc1"], st[u]["acc2"]
                    _, _, _, w = UNITS[u]
                    s1bc = red.tile([P, CH], f32, tag="s1bc", name=f"s1bc{u}", bufs=2)
                    nc.gpsimd.partition_all_reduce(
                        s1bc[:, :w], acc1[:, :w], 128, bass_isa.ReduceOp.add
                    )
                    s2bc = red.tile([P, CH], f32, tag="s2bc", name=f"s2bc{u}", bufs=2)
                    nc.gpsimd.partition_all_reduce(
                        s2bc[:, :w], acc2[:, :w], 128, bass_isa.ReduceOp.add
                    )
                    st[u]["s1bc"], st[u]["s2bc"] = s1bc, s2bc

                def finish_recip(u):
                    if "r1" in st[u]:
                        return
                    _, _, _, w = UNITS[u]
                    # one slot after the partition reduce so the reciprocal
                    # doesn't head-of-line-block DVE while Pool finishes
                    r1 = red.tile([P, CH], f32, tag="r1", name=f"r1_{u}", bufs=1)
                    nc.vector.reciprocal(r1[:, :w], st[u]["s1bc"][:, :w])
                    r2 = red.tile([P, CH], f32, tag="r2", name=f"r2_{u}", bufs=1)
                    nc.vector.reciprocal(r2[:, :w], st[u]["s2bc"][:, :w])
                    st[u]["r1"], st[u]["r2"] = r1, r2

                def emit_pv(u, p):
                    ic, hh, q0, w = UNITS[u]
                    e1, e2 = st[u]["e1"], st[u]["e2"]
                    pv1, pv2 = st[u]["pv1"], st[u]["pv2"]
                    for j in (2 * p, 2 * p + 1):
                        vt = vsb[hh][j // 4][:, j % 4, :]
                        nc.tensor.matmul(
                            pv1[:, :w], lhsT=vt, rhs=e1[:, j, :w],
                            start=(j == 0), stop=(j == JT - 1),
                            skip_group_check=True,
                        )
                        nc.tensor.matmul(
                            pv2[:, :w], lhsT=vt, rhs=e2[:, j, :w],
                            start=(j == 0), stop=(j == JT - 1),
                            skip_group_check=True,
                        )

                def finish_unit(u):
                    ic, hh, q0, w = UNITS[u]
                    pv1, pv2 = st[u]["pv1"], st[u]["pv2"]
                    r1, r2 = st[u]["r1"], st[u]["r2"]
                    t1 = comb.tile([P, CH], f32, tag="t1", name=f"t1_{u}")
                    nc.vector.tensor_tensor(
                        t1[:, :w], pv1[:, :w], r1[:, :w], AluOpType.mult
                    )
                    t2 = comb.tile([P, CH], f32, tag="t2", name=f"t2_{u}")
                    nc.vector.scalar_tensor_tensor(
                        t2[:, :w], pv2[:, :w], nlam_bc, r2[:, :w],
                        AluOpType.mult, AluOpType.mult,
                    )
                    oc = agst.tile([P, CH], bf, tag="oc", name=f"oc{u}")
                    nc.vector.tensor_tensor(
                        oc[:, :w], t1[:, :w], t2[:, :w], AluOpType.add
                    )
                    if ic == NCH - 1 and hh == 1:
                        # head-1 sub-chunk: own staging buffer + small gather
                        s = q0 // HW2
                        nc.sync.dma_start(ag3q_in[s][:, :], oc[:, :w])
                        if collective:
                            nc.gpsimd.collective_compute(
                                "AllGather",
                                AluOpType.bypass,
                                replica_groups=[list(range(NCORES))],
                                ins=[ag3q_in[s][:, :]],
                                outs=[ag_out3q[s][:, :]],
                            )
                        else:
                            nc.sync.dma_start(ag_out3q[s][:P, :], ag3q_in[s][:, :])
                        return
                    nc.sync.dma_start(ag_in[ic][hh * P : (hh + 1) * P, :], oc)
                    if ic == NCH - 1:
                        # head-0 full-width gather: fires 2.5 units early
                        if collective:
                            nc.gpsimd.collective_compute(
                                "AllGather",
                                AluOpType.bypass,
                                replica_groups=[list(range(NCORES))],
                                ins=[ag_in[ic][:P, :]],
                                outs=[ag_out3h0[:, :]],
                            )
                        else:
                            nc.sync.dma_start(ag_out3h0[:P, :], ag_in[ic][:P, :])
                    elif hh == HPC - 1:
                        emit_allgather(ic)

                # filler tasks for early slots: 2 V j-blocks per slot, then
                # the remaining q-chunk projections
                def v_task(nbs):
                    emit_v(nbs, p_pv, "pv")
                    emit_v(nbs + 1, p_pv, "pv")

                def q_task(nci):
                    sq1, sq2 = emit_qk_strip2(0, 1, nci, p_s, "s")
                    emit_rope(sq1, sq2, q1c[nci], q2c[nci], nci, rope_t)

                def k_task(nci):
                    sk1, sk2 = emit_qk_strip2(2, 3, nci, p_s, "s")
                    emit_rope(sk1, sk2, k1c[nci], k2c[nci], nci, rope_t)

                tasks = [
                    lambda: k_task(2),
                    lambda: k_task(3),
                    lambda: v_task(4),
                    lambda: v_task(6),
                    lambda: q_task(1),
                    lambda: v_task(8),
                    lambda: v_task(10),
                    lambda: q_task(2),
                    lambda: v_task(12),
                    lambda: v_task(14),
                    lambda: q_task(3),
                ]

                ACC_LAG, PV_LAG = 1, 4
                total = NU * NPAIR
                proj_at = {(2 * ic + 3): ic for ic in range(NCH - 1)}
                # final-chunk proj handled via emit_proj3_head
                LAST_PV_LAG = 2  # shrink the exposed epilogue of the last unit

                def pv_lag(pair):
                    return LAST_PV_LAG if pair >= (NU - 1) * NPAIR else PV_LAG

                pv_next = 0
                LASTU = (NU - 1) * NPAIR
                for g in range(total + PV_LAG + 1):
                    if tasks and g < 11:
                        tasks.pop(0)()
                    if g < total:
                        u, p = divmod(g, NPAIR)
                        if p == 0:
                            unit_alloc(u)
                            if u in proj_at:
                                emit_proj_dma(proj_at[u])
                        if p == 5 and u in proj_at:
                            emit_proj_mm(proj_at[u])
                        emit_s(u, p)
                    ga = g - ACC_LAG
                    if 0 <= ga < total:
                        u, p = divmod(ga, NPAIR)
                        emit_acc(u, p)
                        if p == NPAIR - 1:
                            finish_acc(u)
                    gr = g - ACC_LAG - 1
                    if 0 <= gr < total:
                        u, p = divmod(gr, NPAIR)
                        if p == NPAIR - 1:
                            finish_recip(u)
                    while pv_next < total and pv_next + pv_lag(pv_next) <= g:
                        u, p = divmod(pv_next, NPAIR)
                        emit_pv(u, p)
                        if p == NPAIR - 1:
                            if u == NU - 1:
                                finish_recip(u)
                            finish_unit(u)
                        pv_next += 1
                    if g == LASTU:
                        emit_proj3h0_dma()
                    elif g == LASTU + 2:
                        emit_proj3h0_mm()
                    elif g == LASTU + 5:
                        emit_proj3h1_dma(0)
                    elif g == total + 1:
                        emit_proj3h1_mm(0)
                emit_proj3h1_dma(1)
                emit_proj3h1_mm(1)
            rope_cm.__exit__(None, None, None)

    nc.compile()
    return nc


def _get_prog():
    global _PROG
    if _PROG is None:
        _PROG = _build_program()
    return _PROG


def _prep_in_maps(x, W_qkv, W_proj, lambda_q1, lambda_q2, lambda_k1, lambda_k2):
    x = np.asarray(x, np.float32).reshape(N, C)
    W_qkv = np.asarray(W_qkv, np.float32)
    W_proj = np.asarray(W_proj, np.float32)

    xT = np.ascontiguousarray(x.T).astype(BF16)

    inv_freq = 1.0 / (10000.0 ** (np.arange(0, DV, 2, dtype=np.float32) / DV))
    freqs = np.arange(N, dtype=np.float32)[:, None] * inv_freq[None, :]  # [N, 64]
    cos = np.cos(freqs).astype(np.float32).T  # [64, N]
    sin = np.sin(freqs).astype(np.float32).T
    cosT = np.concatenate([cos, cos], axis=0)  # [128, N]
    sinT = np.concatenate([sin, sin], axis=0)
    csT = np.concatenate([cosT, sinT], axis=0).astype(BF16)  # [256, N]

    lam_init = 0.8 - 0.6 * float(np.exp(-0.3 * 0.0))
    lam = (
        float(np.exp(np.sum(lambda_q1.astype(np.float32) * lambda_k1.astype(np.float32))))
        - float(np.exp(np.sum(lambda_q2.astype(np.float32) * lambda_k2.astype(np.float32))))
        + lam_init
    )
    nlam = np.full((1, 1), -lam, dtype=np.float32)

    W_projT = np.ascontiguousarray(W_proj.T)  # [2C, C]

    in_maps = []
    for r in range(NCORES):
        hA, hB = 2 * r, 2 * r + 1
        idx_qk = np.concatenate(
            [
                np.arange(g * C + h * HD, g * C + (h + 1) * HD)
                for g in range(4)
                for h in (hA, hB)
            ]
        )
        idx_v = np.concatenate(
            [
                np.arange(g * C + h * HD, g * C + (h + 1) * HD)
                for h in (hA, hB)
                for g in (4, 5)
            ]
        )
        wqk_r = np.ascontiguousarray(W_qkv[idx_qk, :].T).astype(BF16)  # [C, 512]
        wv_r = np.ascontiguousarray(W_qkv[idx_v, :].T).astype(BF16)  # [C, 256]
        wpj_r = np.ascontiguousarray(W_projT[:, r * P : (r + 1) * P]).astype(BF16)
        in_maps.append(
            dict(
                xT=xT,
                wqk=wqk_r,
                wv=wv_r,
                wpj=wpj_r,
                csT=csT,
                nlam=nlam,
            )
        )
    return in_maps


LAST_EXEC_TIME_NS = None


def kernel(x, W_qkv, W_proj, lambda_q1, lambda_q2, lambda_k1, lambda_k2):
    global LAST_EXEC_TIME_NS
    nc = _get_prog()
    in_maps = _prep_in_maps(
        x, W_qkv, W_proj, lambda_q1, lambda_q2, lambda_k1, lambda_k2
    )
    res = run_bass_kernel_spmd(nc, in_maps, core_ids=list(range(NCORES)))
    LAST_EXEC_TIME_NS = res.exec_time_ns
    yT_full = np.concatenate([res.results[r]["yT"] for r in range(NCORES)], axis=0)
    return np.ascontiguousarray(yT_full.T).reshape(B, N, C).astype(np.float32)



# revision 45
# speedup vs baseline: 1.0551x; 1.0551x over previous
"""Differential attention (B=1, N=2048, C=1024, H=16) on 8 Trainium2 NeuronCores.

Sharding: tensor-parallel over heads. Each core owns 2 heads: it computes the
QKV projection for its heads only, runs RoPE + the two softmaxes + PV locally,
then the per-head attention outputs (in transposed [dv, n] layout) are
AllGathered chunk-by-chunk (4 x 256KB/rank, far cheaper than the 8MB
all-reduce alternative and overlapped with the attention tail) and each core
computes a 128-column slice of the output projection.

All matmuls run in bf16 with fp32 PSUM accumulation. Softmax skips
max-subtraction: logits are q.k/8 with q,k ~ N(0,1) so |logit| < ~7 and exp()
is comfortably inside the fp32 range. Denominators are accumulated in bf16 on
the DVE (2-byte fast mode) and reduced across partitions on GPSIMD in fp32.

Schedule notes (cost-model-driven):
- Inputs load via FEW large DMAs: each dma_start costs ~650ns of SP-sequencer
  issue + ~625ns of shared-HWDGE time regardless of size, so per-tile loads
  serialize the prologue on instruction issue, not bytes.
- The PE clock p-state ramps only while the engine stays busy; idle gaps
  restart the following instruction train at 0.65-1.2GHz. The prologue is
  ordered so the PE never starves (v strips interleaved between strip groups,
  filler tasks front-load rope-independent work).
- The final chunk is processed as head-0 full width plus two 256-query head-1
  sub-units, each with its own small AllGather, so the exposed tail after the
  last PV is one quarter-size gather + 8 matmuls instead of a half-chunk
  chain.
"""

import os
import sys

import numpy as np
import ml_dtypes

for _p in ("/opt/trn_rl_repo", os.path.expanduser("~/.axon_site/_ro/trn_rl_repo")):
    if os.path.isdir(_p) and _p not in sys.path:
        sys.path.insert(0, _p)

import concourse.bass as bass  # noqa: E402
import concourse.tile as tile  # noqa: E402
from concourse import bacc, bass_isa, mybir  # noqa: E402
from concourse.alu_op_type import AluOpType  # noqa: E402
from concourse.bass_utils import run_bass_kernel_spmd  # noqa: E402

BF16 = ml_dtypes.bfloat16
B, N, C = 1, 2048, 1024
H = 16
HD = 64  # head dim of each rope/attn half
DV = 2 * HD  # value dim per head (128)
NCORES = 8
HPC = H // NCORES  # heads per core = 2
P = 128
KT = C // P  # 8 contraction tiles for QKV
JT = N // P  # 16 key-position tiles
NCH = 4  # 512-wide chunks of the sequence
CH = N // NCH  # 512
SCALE = HD**-0.5

_PROG = None


def _build_program(collective=True, use_z=False):
    dt = mybir.dt
    f32 = dt.float32
    bf = dt.bfloat16
    Exp = mybir.ActivationFunctionType.Exp

    nc = bacc.Bacc(
        "TRN2", target_bir_lowering=False, debug=False, num_devices=NCORES
    )

    xT = nc.dram_tensor("xT", [C, N], bf, kind="ExternalInput")
    wqk = nc.dram_tensor("wqk", [C, 4 * P], bf, kind="ExternalInput")
    wv = nc.dram_tensor("wv", [C, HPC * DV], bf, kind="ExternalInput")
    wpj = nc.dram_tensor("wpj", [2 * C, P], bf, kind="ExternalInput")
    csT = nc.dram_tensor("csT", [2 * P, N], bf, kind="ExternalInput")
    nlam = nc.dram_tensor("nlam", [1, 1], f32, kind="ExternalInput")
    ident = nc.dram_tensor("ident", [P, P], f32, kind="ExternalInput")
    wpj_rs = nc.dram_tensor("wpj_rs", [2 * P, C], bf, kind="ExternalInput")
    yT = nc.dram_tensor("yT", [P, N], bf, kind="ExternalOutput")

    with tile.TileContext(nc) as tc:
        with tc.tile_pool(name="const", bufs=1) as const, tc.tile_pool(
            name="dram", bufs=1, space="DRAM"
        ) as dram:
            # ---- persistent SBUF tensors ----
            # Inputs are loaded with FEW large DMAs: every dma_start costs
            # ~650ns on the SP sequencer + ~625ns on the shared HWDGE
            # regardless of size, so per-[128,512]-tile loads serialize the
            # prologue on instruction issue, not bytes. Chunk 0 of x and wqk
            # are split into k-pair pieces so the first QKV matmul starts
            # after ~0.5MB; everything else is one DMA per tensor.
            wqk_all = const.tile([P, KT, 4 * P], bf, tag="wqk", name="wqk")
            x0 = const.tile([P, KT, CH], bf, tag="xc0", name="xc0")
            cs_sb = const.tile([P, 2, N], bf, tag="cs", name="cs")
            # micro first pieces: k-tile 0 of the k-strips' weight columns and
            # k-tile 0 of x, so the very first matmul can start ~1us earlier
            nc.sync.dma_start(
                wqk_all[:, 0, 2 * P :],
                wqk.ap()[0:P, 2 * P :].rearrange("(t p) n -> p t n", p=P)[:, 0, :],
            )
            nc.sync.dma_start(
                x0[:, 0, :],
                xT.ap()[0:P, 0:CH].rearrange("(t p) n -> p t n", p=P)[:, 0, :],
            )
            nc.sync.dma_start(
                wqk_all[:, 0, : 2 * P],
                wqk.ap()[0:P, : 2 * P].rearrange("(t p) n -> p t n", p=P)[:, 0, :],
            )
            nc.sync.dma_start(
                wqk_all[:, 1, :],
                wqk.ap()[P : 2 * P, :].rearrange("(t p) n -> p t n", p=P)[:, 0, :],
            )
            nc.sync.dma_start(
                x0[:, 1, :],
                xT.ap()[P : 2 * P, 0:CH].rearrange("(t p) n -> p t n", p=P)[:, 0, :],
            )
            for kp in range(1, 4):
                ks = slice(2 * kp * P, 2 * (kp + 1) * P)
                nc.sync.dma_start(
                    wqk_all[:, 2 * kp : 2 * kp + 2, :],
                    wqk.ap()[ks, :].rearrange("(t p) n -> p t n", p=P),
                )
                nc.sync.dma_start(
                    x0[:, 2 * kp : 2 * kp + 2, :],
                    xT.ap()[ks, 0:CH].rearrange("(t p) n -> p t n", p=P),
                )
            nc.sync.dma_start(
                cs_sb[:, :, 0:CH],
                csT.ap()[:, 0:CH].rearrange("(t p) n -> p t n", p=P),
            )
            wqk_sb = [wqk_all[:, k, :] for k in range(KT)]
            xc = [x0] + [
                const.tile([P, KT, CH], bf, tag=f"xc{c}", name=f"xc{c}")
                for c in range(1, NCH)
            ]
            xsb = [[xc[c][:, k, :] for c in range(NCH)] for k in range(KT)]

            def load_x_chunk(c):
                nc.sync.dma_start(
                    xc[c],
                    xT.ap()[:, c * CH : (c + 1) * CH].rearrange(
                        "(t p) n -> p t n", p=P
                    ),
                )

            cos_sb = cs_sb[:, 0, :]
            sin_sb = cs_sb[:, 1, :]
            wv_all = const.tile([P, KT, HPC * DV], bf, tag="wv", name="wv")
            nc.sync.dma_start(wv_all, wv.ap().rearrange("(t p) n -> p t n", p=P))
            wv_sb = [wv_all[:, k, :] for k in range(KT)]
            load_x_chunk(1)
            nc.sync.dma_start(
                cs_sb[:, :, CH:],
                csT.ap()[:, CH:].rearrange("(t p) n -> p t n", p=P),
            )
            load_x_chunk(2)
            load_x_chunk(3)
            nlam_sb = const.tile([1, 1], f32, tag="nlam")
            nc.sync.dma_start(nlam_sb, nlam.ap())
            nlam_bc = const.tile([P, 1], f32, tag="nlambc")
            nc.gpsimd.partition_broadcast(nlam_bc, nlam_sb)
            wpj_all = const.tile([P, JT, P], bf, tag="wpj", name="wpj")
            nc.sync.dma_start(wpj_all, wpj.ap().rearrange("(t p) n -> p t n", p=P))
            wpj_sb = [wpj_all[:, k, :] for k in range(JT)]
            # ones column for the softmax-denominator matmuls (Z = e.T @ 1)
            # and identity for the PE-transpose of the reciprocals
            wpj_rs_sb = const.tile([P, 2, C], bf, tag="wpjrs", name="wpjrs")
            nc.sync.dma_start(
                wpj_rs_sb, wpj_rs.ap().rearrange("(t p) n -> p t n", p=P)
            )
            ones_sb = const.tile([P, 1], bf, tag="ones")
            nc.vector.memset(ones_sb, 1.0)
            # warm the Exp activation table at t~0 so the first real exp
            # doesn't pay the 1.3us table load on the critical path
            actwarm = const.tile([1, 1], f32, tag="actwarm")
            nc.scalar.activation(actwarm, ones_sb[0:1, :], Exp)
            ident_sb = const.tile([P, P], f32, tag="ident", name="ident")
            nc.sync.dma_start(ident_sb, ident.ap())

            # rope'd projections, [d(2 heads stacked), n], one tile per chunk
            def chunk_tiles(nm):
                return [
                    const.tile([P, CH], bf, tag=f"{nm}{c}", name=f"{nm}{c}")
                    for c in range(NCH)
                ]

            q1c, q2c = chunk_tiles("q1c"), chunk_tiles("q2c")
            k1c, k2c = chunk_tiles("k1c"), chunk_tiles("k2c")
            # values in [j, dv] layout, per head, split into 4 j-groups so
            # PV can start before the whole V projection has finished
            vsb = [
                [
                    const.tile([P, 4, DV], bf, tag=f"vsb{h}_{g}", name=f"vsb{h}_{g}")
                    for g in range(4)
                ]
                for h in range(HPC)
            ]

            # per-chunk AllGather buffers (overlap collectives w/ compute)
            ag_in = [
                dram.tile([HPC * DV, CH], bf, name=f"ag_in{i}") for i in range(NCH)
            ]
            ag_out = [
                dram.tile([2 * C, CH], bf, addr_space="Shared", name=f"ag_out{i}")
                for i in range(NCH)
            ]
            # final chunk gathered in three pieces so the exposed tail after
            # the very last PV is a quarter-size gather + 8 matmuls: head 0
            # full-width (fires two units early), head 1 as two 256-query
            # sub-units
            HW2 = CH // 2
            ag3h0_in = dram.tile([P, HW2], bf, name="ag3h0_in")
            ag_out3h0 = dram.tile([C, HW2], bf, addr_space="Shared", name="ag_out3h0")
            ag3q_in = [dram.tile([P, HW2], bf, name="ag3q_in0")]
            ag_out3q = [
                dram.tile([C, HW2], bf, addr_space="Shared", name="ag_out3q0")
            ]
            # partial-projection staging for the ReduceScatter tail: local
            # heads x all 1024 output columns for the last 256 queries
            rs_in = dram.tile([C, HW2], bf, name="rs_in")
            rs_out = dram.tile([P, HW2], bf, name="rs_out")

            # ---- phase 1 prologue: keys, q-chunk 0, first V j-group ----
            def emit_v(nb, pool, tag):
                c, off = nb // 4, (nb % 4) * P
                pvs = pool.tile([P, HPC * DV], f32, tag=tag, name=f"vstrip{nb}")
                for k in range(KT):
                    nc.tensor.matmul(
                        pvs,
                        lhsT=xsb[k][c][:, off : off + P],
                        rhs=wv_sb[k],
                        start=(k == 0),
                        stop=(k == KT - 1),
                    )
                # copies on DVE: the Activation engine is the per-pair pacer
                # (two exps), so unit-boundary copy bursts must stay off it
                for h in range(HPC):
                    nc.vector.tensor_copy(
                        vsb[h][nb // 4][:, nb % 4, :], pvs[:, h * DV : (h + 1) * DV]
                    )

            def emit_qk_strip2(m1, m2, nci, pool, tag):
                # two strips with the k-loop interleaved, so during the
                # prologue each matmul only waits for its own k-pair DMA
                pt1 = pool.tile([P, CH], f32, tag=tag, name=f"qkstrip{m1}_{nci}")
                pt2 = pool.tile([P, CH], f32, tag=tag, name=f"qkstrip{m2}_{nci}")
                for k in range(KT):
                    nc.tensor.matmul(
                        pt1,
                        lhsT=wqk_sb[k][:, m1 * P : (m1 + 1) * P],
                        rhs=xsb[k][nci],
                        start=(k == 0),
                        stop=(k == KT - 1),
                    )
                    nc.tensor.matmul(
                        pt2,
                        lhsT=wqk_sb[k][:, m2 * P : (m2 + 1) * P],
                        rhs=xsb[k][nci],
                        start=(k == 0),
                        stop=(k == KT - 1),
                    )
                return pt1, pt2

            def emit_rope(s1t, s2t, o1, o2, nci, pool):
                # ops ordered so each PSUM strip is read by exactly the first
                # two DVE ops touching it, releasing its slot early. The four
                # mults read f32 PSUM (1x DVE); intermediates land in bf16
                # SBUF so the two combine ops run in the fast 2-byte mode.
                ns = slice(nci * CH, (nci + 1) * CH)
                a = pool.tile([P, CH], bf, tag="ropetmp", name="rt_a")
                b2 = pool.tile([P, CH], bf, tag="ropetmp", name="rt_b2")
                nc.vector.tensor_tensor(a, s1t, cos_sb[:, ns], AluOpType.mult)
                nc.vector.tensor_tensor(b2, s1t, sin_sb[:, ns], AluOpType.mult)
                b = pool.tile([P, CH], bf, tag="ropetmp", name="rt_b")
                a2 = pool.tile([P, CH], bf, tag="ropetmp", name="rt_a2")
                nc.vector.tensor_tensor(b, s2t, sin_sb[:, ns], AluOpType.mult)
                nc.vector.tensor_tensor(a2, s2t, cos_sb[:, ns], AluOpType.mult)
                nc.vector.tensor_tensor(o1, a, b, AluOpType.subtract)
                nc.vector.tensor_tensor(o2, a2, b2, AluOpType.add)

            rope_cm = tc.tile_pool(name="rope", bufs=6)
            rope_t = rope_cm.__enter__()
            with tc.tile_pool(name="p_pro", bufs=6, space="PSUM") as p_pro, \
                 tc.tile_pool(name="p_prov", bufs=2, space="PSUM") as p_prov:
                # unit-0 pair p only needs key chunk p//2: rope k0/k1 here,
                # push k2/k3 into pipeline filler tasks. V strips are
                # interleaved with the k-c1 strips so the PE never waits on
                # the v-copy (Act) round-trips between p_prov slot reuses.
                sk1, sk2 = emit_qk_strip2(2, 3, 0, p_pro, "prostrip")
                emit_rope(sk1, sk2, k1c[0], k2c[0], 0, rope_t)
                sq1, sq2 = emit_qk_strip2(0, 1, 0, p_pro, "prostrip")
                emit_rope(sq1, sq2, q1c[0], q2c[0], 0, rope_t)
                emit_v(0, p_prov, "vpro")
                emit_v(1, p_prov, "vpro")
                sk1, sk2 = emit_qk_strip2(2, 3, 1, p_pro, "prostrip")
                emit_rope(sk1, sk2, k1c[1], k2c[1], 1, rope_t)
                emit_v(2, p_prov, "vpro")
                emit_v(3, p_prov, "vpro")

            # ---- phase 2: flat attention pipeline + gathered projection ----
            # Remaining V j-groups and q-chunks 1-3 are injected as filler
            # tasks into the early pipeline slots.
            with tc.tile_pool(name="e", bufs=2) as e_pool, \
                 tc.tile_pool(name="nrm", bufs=2) as nrm, \
                 tc.tile_pool(name="comb", bufs=1) as comb, \
                 tc.tile_pool(name="agst", bufs=3) as agst, \
                 tc.tile_pool(name="prhs", bufs=4) as prhs, \
                 tc.tile_pool(name="yst", bufs=1) as yst, \
                 tc.tile_pool(name="p_s", bufs=2, space="PSUM") as p_s, \
                 tc.tile_pool(name="p_z", bufs=1, space="PSUM") as p_z, \
                 tc.tile_pool(name="p_pv", bufs=3, space="PSUM") as p_pv:

                # Persistent 1-bank PSUM scratch for softmax denominators:
                # two 256-col halves used by alternating units. Within a
                # half: cols 0..7 accumulate Z (col = attn*4 + qslice) via
                # free-size-1 matmuls against the ones column; cols 8..135
                # receive the PE-transpose of the reciprocals [8, 128].
                zp_all = p_z.tile([P, 2, 256], f32, tag="zp", name="zp")

                def emit_allgather(ic):
                    if collective:
                        nc.gpsimd.collective_compute(
                            "AllGather",
                            AluOpType.bypass,
                            replica_groups=[list(range(NCORES))],
                            ins=[ag_in[ic][:, :]],
                            outs=[ag_out[ic][:, :]],
                        )
                    else:
                        # timing-only stand-in (single-core TimelineSim)
                        nc.sync.dma_start(ag_out[ic][: HPC * DV, :], ag_in[ic][:, :])

                proj_state = {}

                def emit_proj_dma(ic):
                    rchs = []
                    for g in range(2):
                        rch = prhs.tile(
                            [P, 8, CH], bf, tag="rch", name=f"rch{ic}_{g}", bufs=2
                        )
                        nc.sync.dma_start(
                            rch,
                            ag_out[ic][g * 8 * P : (g + 1) * 8 * P, :].rearrange(
                                "(t p) n -> p t n", p=P
                            ),
                        )
                        rchs.append(rch)
                    proj_state[ic] = rchs

                def emit_proj_mm(ic, part=None):
                    # part=None: all 16 k-tiles; part=0..3: 4 k-tiles each,
                    # so the proj can be metered out as late PE filler
                    parts = range(4) if part is None else [part]
                    if part in (None, 0):
                        proj_state[f"py{ic}"] = p_pv.tile(
                            [P, CH], f32, tag="pv", name=f"py{ic}"
                        )
                    py = proj_state[f"py{ic}"]
                    rchs = proj_state[ic]
                    for pt in parts:
                        for t in range(4):
                            kt = pt * 4 + t
                            nc.tensor.matmul(
                                py, lhsT=wpj_sb[kt], rhs=rchs[kt // 8][:, kt % 8, :],
                                start=(kt == 0), stop=(kt == JT - 1),
                            )
                    if part in (None, 3):
                        proj_state.pop(ic)
                        proj_state.pop(f"py{ic}")
                        ysb = yst.tile([P, CH], bf, tag="ysb", name=f"ysb{ic}")
                        nc.vector.tensor_copy(ysb, py)
                        nc.sync.dma_start(yT.ap()[:, ic * CH : (ic + 1) * CH], ysb)

                # --- final-chunk projection, in three pieces ---
                # rows r*128:(r+1)*128 of a gathered head-h buffer hold head
                # (2r + h) of the concat dim, i.e. global kt = 2r + h
                py3 = [None]
                p3st = {}

                def emit_proj3h0_dma():
                    rch = prhs.tile(
                        [P, KT, HW2], bf, tag="rch3h0", name="rch3h0", bufs=1
                    )
                    nc.sync.dma_start(
                        rch, ag_out3h0[:, :].rearrange("(t p) n -> p t n", p=P)
                    )
                    p3st["h0"] = rch

                def emit_proj3h0_mm():
                    py3[0] = p_pv.tile([P, HW2], f32, tag="pv", name="py3")
                    rch = p3st.pop("h0")
                    for r in range(KT):
                        nc.tensor.matmul(
                            py3[0], lhsT=wpj_sb[2 * r], rhs=rch[:, r, :],
                            start=(r == 0), stop=False,
                            skip_group_check=True,
                        )

                def emit_proj3h1_dma(s):
                    rch = prhs.tile(
                        [P, KT, HW2], bf, tag="rch3q", name=f"rch3q{s}", bufs=1
                    )
                    half = C // 2
                    nc.sync.dma_start(
                        rch[:, : KT // 2, :],
                        ag_out3q[s][:half, :].rearrange("(t p) n -> p t n", p=P),
                    )
                    nc.scalar.dma_start(
                        rch[:, KT // 2 :, :],
                        ag_out3q[s][half:, :].rearrange("(t p) n -> p t n", p=P),
                    )
                    p3st[s] = rch

                def emit_proj3h1_mm(s):
                    py = py3[0]
                    rch = p3st.pop(s)
                    for r in range(KT):
                        nc.tensor.matmul(
                            py, lhsT=wpj_sb[2 * r + 1], rhs=rch[:, r, :],
                            start=False, stop=(r == KT - 1),
                            skip_group_check=True,
                        )
                    ysb = yst.tile([P, HW2], bf, tag="ysb3", name=f"ysb3_{s}")
                    nc.vector.tensor_copy(ysb, py)
                    nc.sync.dma_start(
                        yT.ap()[:, (NCH - 1) * CH + s * HW2 : (NCH - 1) * CH + (s + 1) * HW2],
                        ysb,
                    )

                UNITS = [
                    (ic, hh, 0, CH) for ic in range(NCH - 1) for hh in range(HPC)
                ] + [(3, 0, 0, CH), (3, 1, 0, HW2), (3, 1, HW2, HW2)]
                NU = len(UNITS)
                NPAIR = JT // 2
                st = [None] * NU

                def unit_alloc(u):
                    st[u] = dict(
                        e1=e_pool.tile([P, JT, CH], bf, tag="e1", name=f"e1_{u}"),
                        e2=e_pool.tile([P, JT, CH], bf, tag="e2", name=f"e2_{u}"),
                        pv1=p_pv.tile([P, CH], f32, tag="pv", name=f"pv1_{u}"),
                        pv2=p_pv.tile([P, CH], f32, tag="pv", name=f"pv2_{u}"),
                        zp=zp_all[:, u % 2, :],
                    )

                def emit_s(u, p):
                    ic, hh, q0, w = UNITS[u]
                    hs = slice(HD * hh, HD * (hh + 1))
                    qs = slice(q0, q0 + w)
                    jb = 2 * p
                    c, o0, o1 = jb // 4, (jb % 4) * P, (jb % 4 + 1) * P
                    e1, e2 = st[u]["e1"], st[u]["e2"]
                    s1 = p_s.tile([P, 2, CH], f32, tag="s", name="s1t")
                    nc.tensor.matmul(
                        s1[:, 0, :w], lhsT=k1c[c][hs, o0 : o0 + P],
                        rhs=q1c[ic][hs, qs], start=True, stop=True,
                    )
                    nc.tensor.matmul(
                        s1[:, 1, :w], lhsT=k1c[c][hs, o1 : o1 + P],
                        rhs=q1c[ic][hs, qs], start=True, stop=True,
                    )
                    nc.scalar.activation(
                        e1[:, jb : jb + 2, :w], s1[:, :, :w], Exp, scale=SCALE
                    )
                    s2 = p_s.tile([P, 2, CH], f32, tag="s", name="s2t")
                    nc.tensor.matmul(
                        s2[:, 0, :w], lhsT=k2c[c][hs, o0 : o0 + P],
                        rhs=q2c[ic][hs, qs], start=True, stop=True,
                    )
                    nc.tensor.matmul(
                        s2[:, 1, :w], lhsT=k2c[c][hs, o1 : o1 + P],
                        rhs=q2c[ic][hs, qs], start=True, stop=True,
                    )
                    nc.scalar.activation(
                        e2[:, jb : jb + 2, :w], s2[:, :, :w], Exp, scale=SCALE
                    )

                def emit_z(u, p):
                    # softmax denominators on the PE: Z[q] = sum_j e[j, q]
                    # as free-size-1 matmuls (lhsT = 128-query slice of the
                    # e tile, rhs = ones column), accumulated across the 16
                    # j-tiles in PSUM. Output partitions = queries.
                    ic, hh, q0, w = UNITS[u]
                    e1, e2 = st[u]["e1"], st[u]["e2"]
                    zp = st[u]["zp"]
                    for a, et in ((0, e1), (1, e2)):
                        for s in range(w // P):
                            qs = slice(q0 + s * P, q0 + (s + 1) * P)
                            c = 4 * a + s
                            for j in (2 * p, 2 * p + 1):
                                nc.tensor.matmul(
                                    zp[:, c : c + 1], lhsT=et[:, j, qs],
                                    rhs=ones_sb,
                                    start=(j == 0), stop=(j == JT - 1),
                                    skip_group_check=True,
                                )

                def finish_z_a(u):
                    # Z [128q, 8] -> reciprocals -> fold -lambda into the
                    # attn2 columns (all on DVE)
                    _, _, q0, w = UNITS[u]
                    nsl = w // P
                    zp = st[u]["zp"]
                    rsb = nrm.tile([P, 8], f32, tag="rsb", name=f"rsb{u}")
                    nc.vector.reciprocal(rsb, zp[:, 0:8])
                    nc.vector.tensor_scalar(
                        rsb[:, 4 : 4 + nsl], rsb[:, 4 : 4 + nsl], nlam_bc, None,
                        AluOpType.mult,
                    )
                    st[u]["rsb"] = rsb

                def finish_z_b(u):
                    # PE-transpose the reciprocals to [8, 128], then one
                    # SBUF->SBUF dma folds the 8 rows into a single
                    # partition-0 row (partition_broadcast can only source
                    # partition 0), then broadcast each 128-query stretch.
                    # The dma latency hides under the extra pv lag of the
                    # next unit's first two pairs.
                    if "r1bc" in st[u]:
                        return
                    _, _, q0, w = UNITS[u]
                    nsl = w // P
                    zp = st[u]["zp"]
                    nc.tensor.transpose(zp[0:8, 8:136], st[u]["rsb"], ident_sb)
                    rt = nrm.tile([8, P], f32, tag="rt", name=f"rt{u}")
                    nc.vector.tensor_copy(rt, zp[0:8, 8:136])
                    rtf = nrm.tile([1, 8, P], f32, tag="rtf", name=f"rtf{u}", bufs=1)
                    nc.sync.dma_start(rtf, rt[0:8, :])
                    r1bc = nrm.tile([P, CH], f32, tag="r1bc", name=f"r1bc{u}")
                    r2bc = nrm.tile([P, CH], f32, tag="r2bc", name=f"r2bc{u}")
                    for s in range(nsl):
                        nc.gpsimd.partition_broadcast(
                            r1bc[:, s * P : (s + 1) * P], rtf[0:1, s, :]
                        )
                    for s in range(nsl):
                        nc.gpsimd.partition_broadcast(
                            r2bc[:, s * P : (s + 1) * P], rtf[0:1, 4 + s, :]
                        )
                    st[u]["r1bc"], st[u]["r2bc"] = r1bc, r2bc

                def emit_pv(u, p):
                    ic, hh, q0, w = UNITS[u]
                    e1, e2 = st[u]["e1"], st[u]["e2"]
                    pv1, pv2 = st[u]["pv1"], st[u]["pv2"]
                    for j in (2 * p, 2 * p + 1):
                        vt = vsb[hh][j // 4][:, j % 4, :]
                        nc.tensor.matmul(
                            pv1[:, :w], lhsT=vt, rhs=e1[:, j, :w],
                            start=(j == 0), stop=(j == JT - 1),
                            skip_group_check=True,
                        )
                        nc.tensor.matmul(
                            pv2[:, :w], lhsT=vt, rhs=e2[:, j, :w],
                            start=(j == 0), stop=(j == JT - 1),
                            skip_group_check=True,
                        )

                def emit_acc_last(u, p):
                    # last unit: classic denominator path. DVE and Pool are
                    # idle at the tail and partition_all_reduce natively
                    # produces the broadcast layout, so the r-chain after the
                    # final exp is much shorter than the z/transpose path.
                    _, _, q0, w = UNITS[u]
                    e1, e2 = st[u]["e1"], st[u]["e2"]
                    jb = 2 * p
                    if p == 0:
                        acc1 = nrm.tile([P, CH], bf, tag="acc1", name="acc1l", bufs=2)
                        acc2 = nrm.tile([P, CH], bf, tag="acc2", name="acc2l", bufs=2)
                        st[u]["acc1"], st[u]["acc2"] = acc1, acc2
                        nc.vector.tensor_tensor(
                            acc1[:, :w], e1[:, 0, :w], e1[:, 1, :w], AluOpType.add
                        )
                        nc.vector.tensor_tensor(
                            acc2[:, :w], e2[:, 0, :w], e2[:, 1, :w], AluOpType.add
                        )
                    else:
                        acc1, acc2 = st[u]["acc1"], st[u]["acc2"]
                        for j in (jb, jb + 1):
                            nc.vector.tensor_tensor(
                                acc1[:, :w], acc1[:, :w], e1[:, j, :w], AluOpType.add
                            )
                            nc.vector.tensor_tensor(
                                acc2[:, :w], acc2[:, :w], e2[:, j, :w], AluOpType.add
                            )

                def finish_acc_last(u):
                    _, _, q0, w = UNITS[u]
                    r1bc = nrm.tile([P, CH], f32, tag="r1bc", name="r1bcl")
                    r2bc = nrm.tile([P, CH], f32, tag="r2bc", name="r2bcl")
                    nc.gpsimd.partition_all_reduce(
                        r1bc[:, :w], st[u]["acc1"][:, :w], 128, bass_isa.ReduceOp.add
                    )
                    nc.gpsimd.partition_all_reduce(
                        r2bc[:, :w], st[u]["acc2"][:, :w], 128, bass_isa.ReduceOp.add
                    )
                    nc.vector.reciprocal(r1bc[:, :w], r1bc[:, :w])
                    nc.vector.reciprocal(r2bc[:, :w], r2bc[:, :w])
                    nc.vector.tensor_scalar(
                        r2bc[:, :w], r2bc[:, :w], nlam_bc, None, AluOpType.mult
                    )
                    st[u]["r1bc"], st[u]["r2bc"] = r1bc, r2bc

                def finish_unit(u):
                    ic, hh, q0, w = UNITS[u]
                    pv1, pv2 = st[u]["pv1"], st[u]["pv2"]
                    r1bc, r2bc = st[u]["r1bc"], st[u]["r2bc"]
                    t1 = comb.tile([P, CH], f32, tag="t1", name=f"t1_{u}")
                    nc.vector.tensor_tensor(
                        t1[:, :w], pv1[:, :w], r1bc[:, :w], AluOpType.mult
                    )
                    t2 = comb.tile([P, CH], f32, tag="t2", name=f"t2_{u}")
                    nc.vector.tensor_tensor(
                        t2[:, :w], pv2[:, :w], r2bc[:, :w], AluOpType.mult
                    )
                    oc = agst.tile([P, CH], bf, tag="oc", name=f"oc{u}")
                    nc.vector.tensor_tensor(
                        oc[:, :w], t1[:, :w], t2[:, :w], AluOpType.add
                    )
                    st[u]["oc"] = oc

                def emit_oc_stage(u):
                    # staging dma + gather, emitted ~2 pairs after the combine
                    # so the SP sequencer never parks on the oc semaphore with
                    # urgent proj loads queued behind it
                    ic, hh, q0, w = UNITS[u]
                    oc = st[u]["oc"]
                    if ic == NCH - 1 and hh == 1:
                        if q0 != 0:
                            return  # last piece goes through the RS tail
                        # head-1 first sub-chunk: own staging buffer + small
                        # gather, with the proj reload chained right behind so
                        # the scheduler keeps the whole piece chain adjacent
                        s = 0
                        nc.sync.dma_start(ag3q_in[s][:, :], oc[:, :w])
                        if collective:
                            nc.gpsimd.collective_compute(
                                "AllGather",
                                AluOpType.bypass,
                                replica_groups=[list(range(NCORES))],
                                ins=[ag3q_in[s][:, :]],
                                outs=[ag_out3q[s][:, :]],
                            )
                        else:
                            nc.sync.dma_start(ag_out3q[s][:P, :], ag3q_in[s][:, :])
                        emit_proj3h1_dma(s)
                        return
                    if ic == NCH - 1:
                        # head-0: only the first query half goes through the
                        # gathered projection; the second half feeds the RS
                        # tail straight from SBUF
                        nc.sync.dma_start(ag3h0_in[:, :], oc[:, :HW2])
                        if collective:
                            nc.gpsimd.collective_compute(
                                "AllGather",
                                AluOpType.bypass,
                                replica_groups=[list(range(NCORES))],
                                ins=[ag3h0_in[:, :]],
                                outs=[ag_out3h0[:, :]],
                            )
                        else:
                            nc.sync.dma_start(ag_out3h0[:P, :], ag3h0_in[:, :])
                        emit_proj3h0_dma()
                        return
                    nc.sync.dma_start(ag_in[ic][hh * P : (hh + 1) * P, :], oc)
                    if hh == HPC - 1:
                        emit_allgather(ic)

                # filler tasks for early slots: 2 V j-blocks per slot, then
                # the remaining q-chunk projections
                def v_task(nbs):
                    emit_v(nbs, p_pv, "pv")
                    emit_v(nbs + 1, p_pv, "pv")

                def emit_qk_strip_seq(m1, m2, nci, pool, tag):
                    # post-prologue variant: strips sequential, not
                    # interleaved, so the first strip's PSUM slot is released
                    # to the s-rotation ~1.7us earlier
                    pt1 = pool.tile([P, CH], f32, tag=tag, name=f"qks{m1}_{nci}")
                    for k in range(KT):
                        nc.tensor.matmul(
                            pt1, lhsT=wqk_sb[k][:, m1 * P : (m1 + 1) * P],
                            rhs=xsb[k][nci], start=(k == 0), stop=(k == KT - 1),
                        )
                    pt2 = pool.tile([P, CH], f32, tag=tag, name=f"qks{m2}_{nci}")
                    for k in range(KT):
                        nc.tensor.matmul(
                            pt2, lhsT=wqk_sb[k][:, m2 * P : (m2 + 1) * P],
                            rhs=xsb[k][nci], start=(k == 0), stop=(k == KT - 1),
                        )
                    return pt1, pt2

                def q_task(nci):
                    sq1, sq2 = emit_qk_strip_seq(0, 1, nci, p_s, "s")
                    emit_rope(sq1, sq2, q1c[nci], q2c[nci], nci, rope_t)

                def k_task(nci):
                    sk1, sk2 = emit_qk_strip_seq(2, 3, nci, p_s, "s")
                    emit_rope(sk1, sk2, k1c[nci], k2c[nci], nci, rope_t)

                # filler tasks keyed by the pair slot they are emitted at:
                # k/v strips early (hard deadlines), q-chunk projections and
                # the output projections spread through the ACT-paced middle
                # where the PE otherwise runs out of work
                tasks_at = {
                    0: lambda: k_task(2),
                    1: lambda: k_task(3),
                    2: lambda: v_task(4),
                    3: lambda: v_task(6),
                    5: lambda: v_task(8),
                    6: lambda: v_task(10),
                    8: lambda: v_task(12),
                    9: lambda: v_task(14),
                    12: lambda: q_task(1),
                    20: lambda: q_task(2),
                    36: lambda: q_task(3),
                }

                Z_LAG, PV_LAG = 2, 4
                total = NU * NPAIR
                proj_at = {(2 * ic + 3): ic for ic in range(NCH - 1)}
                # final-chunk proj handled via emit_proj3_head
                LAST_PV_LAG = 2  # shrink the exposed epilogue of the last unit

                def pv_lag(pair):
                    if pair >= (NU - 1) * NPAIR:
                        return LAST_PV_LAG
                    # first two PV pairs of a unit wait 2 extra slots so their
                    # PSUM-slot reuse never races the previous unit's combine
                    return 6 if pair % NPAIR < 2 else PV_LAG

                def emit_rs_tail():
                    # Last 256 queries of the final chunk: instead of
                    # allgather + 16-ktile reload + proj, project the two
                    # local heads' combined outputs onto ALL 1024 output
                    # columns (16 matmuls against the wpj_rs slice) and
                    # ReduceScatter the partial sums straight into yT.
                    # 2 hops instead of 3 and no 1MB reload on the tail.
                    oc_h0 = st[NU - 3]["oc"]
                    oc_h1 = st[NU - 1]["oc"]
                    yps = []
                    for half in range(2):
                        yp = p_s.tile(
                            [P, 4, HW2], f32, tag="s", name=f"yp{half}"
                        )
                        for ct in range(4):
                            c = half * 4 + ct
                            for h, rhs in ((0, oc_h0[:, HW2:]), (1, oc_h1[:, :HW2])):
                                nc.tensor.matmul(
                                    yp[:, ct, :],
                                    lhsT=wpj_rs_sb[:, h, c * P : (c + 1) * P],
                                    rhs=rhs,
                                    start=(h == 0), stop=(h == 1),
                                    skip_group_check=True,
                                )
                        yps.append(yp)
                    ysb_rs = yst.tile([P, 8, HW2], bf, tag="ysbrs", name="ysb_rs")
                    nc.vector.tensor_copy(ysb_rs[:, 0:4, :], yps[0])
                    nc.scalar.copy(ysb_rs[:, 4:8, :], yps[1])
                    nc.sync.dma_start(
                        rs_in[:, :].rearrange("(t p) n -> p t n", p=P), ysb_rs
                    )
                    if collective:
                        nc.gpsimd.collective_compute(
                            "ReduceScatter",
                            AluOpType.add,
                            replica_groups=[list(range(NCORES))],
                            ins=[rs_in[:, :]],
                            outs=[rs_out[:, :]],
                        )
                    else:
                        nc.sync.dma_start(rs_out[:, :], rs_in[0:P, :])
                    nc.sync.dma_start(yT.ap()[:, N - HW2 :], rs_out[:, :])

                pv_next = 0
                LASTU = (NU - 1) * NPAIR
                for g in range(total + PV_LAG + 1):
                    if g in tasks_at:
                        tasks_at.pop(g)()
                    if g < total:
                        u, p = divmod(g, NPAIR)
                        if p == 0:
                            unit_alloc(u)
                            if u in proj_at:
                                emit_proj_dma(proj_at[u])
                        # chunks 0/1: proj matmuls metered 4-at-a-time as
                        # filler over pairs 2..5; chunk 2's matmuls run in
                        # the final-gather wait gap instead (see tail below)
                        if 2 <= p <= 5 and u in proj_at and proj_at[u] != NCH - 2:
                            emit_proj_mm(proj_at[u], part=p - 2)
                        emit_s(u, p)
                    # the last unit's PE has lots of slack (half-width
                    # matmuls) and its r-chain is tail-critical: run its z
                    # matmuls at lag 1 and finish the chain immediately
                    gz = g - Z_LAG
                    if 0 <= gz < (NU - 1) * NPAIR:
                        u, p = divmod(gz, NPAIR)
                        if use_z:
                            emit_z(u, p)
                            if p == NPAIR - 1:
                                finish_z_a(u)
                        else:
                            emit_acc_last(u, p)
                            if p == NPAIR - 1:
                                finish_acc_last(u)
                    gzl = g - 1
                    if (NU - 1) * NPAIR <= gzl < total:
                        u, p = divmod(gzl, NPAIR)
                        emit_acc_last(u, p)
                        if p == NPAIR - 1:
                            finish_acc_last(u)
                    gzb = g - Z_LAG - 1
                    if use_z and 0 <= gzb < (NU - 1) * NPAIR:
                        u, p = divmod(gzb, NPAIR)
                        if p == NPAIR - 1:
                            finish_z_b(u)
                    while pv_next < total and pv_next + pv_lag(pv_next) <= g:
                        u, p = divmod(pv_next, NPAIR)
                        emit_pv(u, p)
                        if p == NPAIR - 1:
                            finish_unit(u)
                            if u == NU - 1:
                                emit_oc_stage(u)
                        pv_next += 1
                    go = g - 13
                    if go >= 0 and go % NPAIR == 0 and go // NPAIR < NU - 1:
                        emit_oc_stage(go // NPAIR)
                    if g == total + 1:
                        emit_proj3h0_mm()
                    elif g == total + 2:
                        emit_proj3h1_mm(0)
                    elif g == total + 3:
                        emit_rs_tail()
                    elif g == total + 4:
                        # tail filler: keeps the PE warm while the RS staging
                        # dmas are in flight
                        emit_proj_mm(NCH - 2)
            rope_cm.__exit__(None, None, None)

    nc.compile()
    return nc


def _get_prog():
    global _PROG
    if _PROG is None:
        _PROG = _build_program()
    return _PROG


def _prep_in_maps(x, W_qkv, W_proj, lambda_q1, lambda_q2, lambda_k1, lambda_k2):
    x = np.asarray(x, np.float32).reshape(N, C)
    W_qkv = np.asarray(W_qkv, np.float32)
    W_proj = np.asarray(W_proj, np.float32)

    xT = np.ascontiguousarray(x.T).astype(BF16)

    inv_freq = 1.0 / (10000.0 ** (np.arange(0, DV, 2, dtype=np.float32) / DV))
    freqs = np.arange(N, dtype=np.float32)[:, None] * inv_freq[None, :]  # [N, 64]
    cos = np.cos(freqs).astype(np.float32).T  # [64, N]
    sin = np.sin(freqs).astype(np.float32).T
    cosT = np.concatenate([cos, cos], axis=0)  # [128, N]
    sinT = np.concatenate([sin, sin], axis=0)
    csT = np.concatenate([cosT, sinT], axis=0).astype(BF16)  # [256, N]

    lam_init = 0.8 - 0.6 * float(np.exp(-0.3 * 0.0))
    lam = (
        float(np.exp(np.sum(lambda_q1.astype(np.float32) * lambda_k1.astype(np.float32))))
        - float(np.exp(np.sum(lambda_q2.astype(np.float32) * lambda_k2.astype(np.float32))))
        + lam_init
    )
    nlam = np.full((1, 1), -lam, dtype=np.float32)

    W_projT = np.ascontiguousarray(W_proj.T)  # [2C, C]

    in_maps = []
    for r in range(NCORES):
        hA, hB = 2 * r, 2 * r + 1
        idx_qk = np.concatenate(
            [
                np.arange(g * C + h * HD, g * C + (h + 1) * HD)
                for g in range(4)
                for h in (hA, hB)
            ]
        )
        idx_v = np.concatenate(
            [
                np.arange(g * C + h * HD, g * C + (h + 1) * HD)
                for h in (hA, hB)
                for g in (4, 5)
            ]
        )
        wqk_r = np.ascontiguousarray(W_qkv[idx_qk, :].T).astype(BF16)  # [C, 512]
        wv_r = np.ascontiguousarray(W_qkv[idx_v, :].T).astype(BF16)  # [C, 256]
        wpj_r = np.ascontiguousarray(W_projT[:, r * P : (r + 1) * P]).astype(BF16)
        wpj_rs_r = np.ascontiguousarray(
            W_projT[r * 2 * P : (r + 1) * 2 * P, :]
        ).astype(BF16)
        in_maps.append(
            dict(
                xT=xT,
                wqk=wqk_r,
                wv=wv_r,
                wpj=wpj_r,
                wpj_rs=wpj_rs_r,
                csT=csT,
                nlam=nlam,
                ident=np.eye(P, dtype=np.float32),
            )
        )
    return in_maps


LAST_EXEC_TIME_NS = None


def kernel(x, W_qkv, W_proj, lambda_q1, lambda_q2, lambda_k1, lambda_k2):
    global LAST_EXEC_TIME_NS
    nc = _get_prog()
    in_maps = _prep_in_maps(
        x, W_qkv, W_proj, lambda_q1, lambda_q2, lambda_k1, lambda_k2
    )
    res = run_bass_kernel_spmd(nc, in_maps, core_ids=list(range(NCORES)))
    LAST_EXEC_TIME_NS = res.exec_time_ns
    yT_full = np.concatenate(
        [np.asarray(res.results[r]["yT"], dtype=np.float32) for r in range(NCORES)],
        axis=0,
    )
    return np.ascontiguousarray(yT_full.T).reshape(B, N, C)

